# revision 4
# baseline (speedup 1.0000x reference)
"""MoE-LoRA double GEMM on 8 Trainium2 NeuronCores.

Computes, for E=4 experts:  h_e = x @ A_e^T ; y_e = h_e @ B_e^T
with x:[4,2048,4096] f32, A:[4,64,4096], B:[4,4096,64] ->
y:[4,4,2048,4096].

Strategy: data-parallel shard x over tokens (8192 tokens -> 1024/core),
replicate the small expert weights. Per core:
  1. PE-transpose x tiles to get x^T (contraction dim D on partitions).
  2. GEMM1 (fp32r): h^T[pair] = [A_2p^T | A_2p+1^T]^T-stacked @ x^T,
     two experts packed along the stationary M axis (r_e=64 each).
  3. GEMM2 (fp32r): y_e tile [128 tok, 512 out] = h_e^T-chunk (stationary,
     K=64) x B_e^T (moving), which yields y in natural [token, out] layout
     for fully-contiguous DMA stores.
PSUM->SBUF copies alternate VectorE/ScalarE so neither becomes the
bottleneck under the ~64 MiB/core output stream.
"""

import os
import sys

import numpy as np

for _p in ("/opt/trn_rl_repo", "/root/.axon_site/_ro/trn_rl_repo"):
    if os.path.isdir(_p) and _p not in sys.path:
        sys.path.append(_p)

from concourse import bacc, mybir, tile
from concourse.bass_utils import run_bass_kernel_spmd
from concourse.masks import make_identity

E = 4
R_E = 64
D = 4096
O = 4096
B_DIM = 4
S = 2048
T = B_DIM * S          # 8192 tokens total
NCORES = 8
TL = T // NCORES       # 1024 tokens per core
TT = 256               # tokens per pipeline tile
NTT = TL // TT         # 4
TG = TT // 128         # 2 token-groups of 128 per tile
NCD = D // 128         # 32 contraction chunks
OC_W = 512             # output columns per matmul (one PSUM bank, fp32)
NOC = O // OC_W        # 8

FP32 = mybir.dt.float32
FP32R = mybir.dt.float32r

_CACHE = {}


def _build_nc():
    nc = bacc.Bacc(None, target_bir_lowering=False, debug=False)
    x_d = nc.declare_dram_parameter("x", [TL, D], FP32, isOutput=False)
    at_d = nc.declare_dram_parameter("AT", [2, NCD, 128, 128], FP32R, isOutput=False)
    bt_d = nc.declare_dram_parameter("BT", [2, 128, O], FP32R, isOutput=False)
    y_d = nc.declare_dram_parameter("y", [E, TL, O], FP32, isOutput=True)

    with tile.TileContext(nc) as tc:
        with (
            tc.tile_pool(name="w", bufs=1) as wpool,
            tc.tile_pool(name="xn", bufs=4) as xnpool,
            tc.tile_pool(name="xt", bufs=1) as xtpool,
            tc.tile_pool(name="ht", bufs=4) as htpool,
            tc.tile_pool(name="ys", bufs=8) as yspool,
            tc.tile_pool(name="ps_xt", bufs=2, space="PSUM") as ps_xt,
            tc.tile_pool(name="ps_ht", bufs=3, space="PSUM") as ps_ht,
            tc.tile_pool(name="ps_y", bufs=3, space="PSUM") as ps_y,
        ):
            ident = wpool.tile([128, 128], FP32)
            make_identity(nc, ident[:])

            at_sb = wpool.tile([128, 2, NCD, 128], FP32R)
            bt_sb = wpool.tile([128, 2, O], FP32R)
            for p in range(2):
                nc.sync.dma_start(out=bt_sb[:, p, :], in_=bt_d[p])
                for c in range(NCD):
                    nc.sync.dma_start(out=at_sb[:, p, c, :], in_=at_d[p, c])

            alt = [0]

            def copy_psum(dst, src):
                if alt[0] % 2 == 0:
                    nc.vector.tensor_copy(dst, src)
                else:
                    nc.scalar.copy(dst, src)
                alt[0] += 1

            for tt in range(NTT):
                t0 = tt * TT
                xn = []
                for g in range(TG):
                    xg = xnpool.tile([128, D], FP32)
                    nc.sync.dma_start(
                        out=xg[:], in_=x_d[t0 + g * 128 : t0 + (g + 1) * 128, :]
                    )
                    xn.append(xg)

                xt = xtpool.tile([128, NCD, TT], FP32R)
                for c in range(NCD):
                    pxt = ps_xt.tile([128, TT], FP32)
                    for g in range(TG):
                        nc.tensor.transpose(
                            pxt[:, g * 128 : (g + 1) * 128],
                            xn[g][:, c * 128 : (c + 1) * 128],
                            ident[:],
                        )
                    copy_psum(xt[:, c, :], pxt[:])

                hts = []
                for p in range(2):
                    pht = ps_ht.tile([128, TT], FP32)
                    for c in range(NCD):
                        nc.tensor.matmul(
                            pht[:],
                            at_sb[:, p, c, :],
                            xt[:, c, :],
                            start=(c == 0),
                            stop=(c == NCD - 1),
                        )
                    ht = htpool.tile([128, TT], FP32R)
                    copy_psum(ht[:], pht[:])
                    hts.append(ht)

                for p in range(2):
                    for s_i in range(2):
                        e = 2 * p + s_i
                        r0 = 64 * s_i
                        for g in range(TG):
                            for oc in range(NOC):
                                py = ps_y.tile([128, OC_W], FP32)
                                nc.tensor.matmul(
                                    py[:],
                                    hts[p][
                                        r0 : r0 + 64, g * 128 : (g + 1) * 128
                                    ],
                                    bt_sb[
                                        r0 : r0 + 64, p, oc * OC_W : (oc + 1) * OC_W
                                    ],
                                    start=True,
                                    stop=True,
                                )
                                ys = yspool.tile([128, OC_W], FP32)
                                copy_psum(ys[:], py[:])
                                nc.sync.dma_start(
                                    out=y_d[
                                        e,
                                        t0 + g * 128 : t0 + (g + 1) * 128,
                                        oc * OC_W : (oc + 1) * OC_W,
                                    ],
                                    in_=ys[:],
                                )
    nc.compile()
    return nc


def _get_nc():
    if "nc" not in _CACHE:
        _CACHE["nc"] = _build_nc()
    return _CACHE["nc"]


def _prep_weights(A, B):
    A = np.asarray(A, dtype=np.float32)
    B = np.asarray(B, dtype=np.float32)
    at = np.empty((2, NCD, 128, 128), dtype=np.float32)
    bt = np.empty((2, 128, O), dtype=np.float32)
    for p in range(2):
        # stationary for GEMM1: [D, 128] with expert 2p in cols 0-63, 2p+1 in 64-127
        atp = np.concatenate([A[2 * p].T, A[2 * p + 1].T], axis=1)  # [4096, 128]
        at[p] = atp.reshape(NCD, 128, 128)
        # moving for GEMM2: [128, O] with expert 2p rows 0-63, 2p+1 rows 64-127
        bt[p] = np.concatenate([B[2 * p].T, B[2 * p + 1].T], axis=0)
    return at, bt


def kernel(x, A, B, _trace=False):
    x = np.asarray(x, dtype=np.float32)
    x_flat = np.ascontiguousarray(x.reshape(T, D))
    at, bt = _prep_weights(A, B)

    nc = _get_nc()
    in_maps = [
        {
            "x": np.ascontiguousarray(x_flat[k * TL : (k + 1) * TL]),
            "AT": at,
            "BT": bt,
        }
        for k in range(NCORES)
    ]
    res = run_bass_kernel_spmd(nc, in_maps, list(range(NCORES)), trace=_trace)
    if _trace:
        _CACHE["last_result"] = res

    y = np.empty((E, T, O), dtype=np.float32)
    for k in range(NCORES):
        y[:, k * TL : (k + 1) * TL, :] = res.results[k]["y"]
    return y.reshape(E, B_DIM, S, O)


# revision 8
# speedup vs baseline: 1.1312x; 1.1312x over previous
"""MoE-LoRA double GEMM on 8 Trainium2 NeuronCores.

Computes, for E=4 experts:  h_e = x @ A_e^T ; y_e = h_e @ B_e^T
with x:[4,2048,4096] f32, A:[4,64,4096], B:[4,4096,64] ->
y:[4,4,2048,4096].

Strategy: data-parallel shard x over tokens (8192 tokens -> 1024/core),
replicate the small expert weights. Host prepares matmul-native layouts
(x^T with the contraction dim D leading, A/B transposed + expert-pair
packed) so the device runs a single dense fp32r matmul stream:
  GEMM1: h^T[pair] = [A_2p^T | A_2p+1^T] (stationary, experts packed on
         the M axis) x x^T tile (moving, N=512) accumulated over D.
  GEMM2: y_e tile [128 tok, 512 out] = h_e^T chunk (stationary, K=64,
         the two experts of a pair issued back-to-back on row strips
         0/64 so they run concurrently in the PE array) x B_e^T
         (moving), giving y in natural [token, out] layout for
         contiguous DMA stores.
PSUM->SBUF copies alternate VectorE/ScalarE so neither engine becomes
the bottleneck under the ~64 MiB/core output stream; the kernel is
HBM-bandwidth bound (~92 MB/core).
"""

import os
import sys

import numpy as np

for _p in ("/opt/trn_rl_repo", "/root/.axon_site/_ro/trn_rl_repo"):
    if os.path.isdir(_p) and _p not in sys.path:
        sys.path.append(_p)

from concourse import bacc, mybir, tile
from concourse.bass_utils import run_bass_kernel_spmd

E = 4
R_E = 64
D = 4096
O = 4096
B_DIM = 4
S = 2048
T = B_DIM * S          # 8192 tokens total
NCORES = 8
TL = T // NCORES       # 1024 tokens per core
TT = 512               # tokens per pipeline tile
NTT = TL // TT         # 2
TG = TT // 128         # 4 token-groups of 128 per tile
NCD = D // 128         # 32 contraction chunks
OC_W = 512             # output columns per matmul (one PSUM bank, fp32)
NOC = O // OC_W        # 8

FP32 = mybir.dt.float32
FP32R = mybir.dt.float32r

_CACHE = {}


def _build_nc():
    nc = bacc.Bacc(None, target_bir_lowering=False, debug=False)
    xt_d = nc.declare_dram_parameter("xT", [D, TL], FP32R, isOutput=False)
    at_d = nc.declare_dram_parameter("AT", [2, NCD, 128, 128], FP32R, isOutput=False)
    bt_d = nc.declare_dram_parameter("BT", [2, 128, O], FP32R, isOutput=False)
    y_d = nc.declare_dram_parameter("y", [E, TL, O], FP32, isOutput=True)

    xt_r = xt_d.rearrange("(c p) t -> p c t", p=128)

    with tile.TileContext(nc) as tc:
        with (
            tc.tile_pool(name="w", bufs=1) as wpool,
            tc.tile_pool(name="xt", bufs=34) as xtpool,
            tc.tile_pool(name="ht", bufs=4) as htpool,
            tc.tile_pool(name="ys", bufs=8) as yspool,
            tc.tile_pool(name="ps_ht", bufs=3, space="PSUM") as ps_ht,
            tc.tile_pool(name="ps_y", bufs=4, space="PSUM") as ps_y,
        ):
            at_sb = wpool.tile([128, 2, NCD, 128], FP32R)
            bt_sb = wpool.tile([128, 2, O], FP32R)
            for p in range(2):
                nc.sync.dma_start(out=bt_sb[:, p, :], in_=bt_d[p])
                for c in range(NCD):
                    nc.sync.dma_start(out=at_sb[:, p, c, :], in_=at_d[p, c])

            alt = [0]

            def copy_psum(dst, src):
                if alt[0] % 2 == 0:
                    nc.vector.tensor_copy(dst, src)
                else:
                    nc.scalar.copy(dst, src)
                alt[0] += 1

            for tt in range(NTT):
                t0 = tt * TT
                xtc = []
                for c in range(NCD):
                    xc = xtpool.tile([128, TT], FP32R, tag="xtc")
                    nc.sync.dma_start(out=xc[:], in_=xt_r[:, c, t0 : t0 + TT])
                    xtc.append(xc)

                phts = [ps_ht.tile([128, TT], FP32, name=f"pht{_p}", tag="pht") for _p in range(2)]
                for c in range(NCD):
                    for p in range(2):
                        nc.tensor.matmul(
                            phts[p][:],
                            at_sb[:, p, c, :],
                            xtc[c][:],
                            start=(c == 0),
                            stop=(c == NCD - 1),
                        )
                hts = []
                for p in range(2):
                    ht = htpool.tile([128, TT], FP32R)
                    copy_psum(ht[:], phts[p][:])
                    hts.append(ht)

                for p in range(2):
                    for g in range(TG):
                        for oc in range(NOC):
                            for s_i in range(2):
                                e = 2 * p + s_i
                                r0 = 64 * s_i
                                py = ps_y.tile([128, OC_W], FP32)
                                nc.tensor.matmul(
                                    py[:],
                                    hts[p][r0 : r0 + 64, g * 128 : (g + 1) * 128],
                                    bt_sb[r0 : r0 + 64, p, oc * OC_W : (oc + 1) * OC_W],
                                    start=True,
                                    stop=True,
                                )
                                ys = yspool.tile([128, OC_W], FP32)
                                copy_psum(ys[:], py[:])
                                nc.sync.dma_start(
                                    out=y_d[
                                        e,
                                        t0 + g * 128 : t0 + (g + 1) * 128,
                                        oc * OC_W : (oc + 1) * OC_W,
                                    ],
                                    in_=ys[:],
                                )
    nc.compile()
    return nc


def _get_nc():
    if "nc" not in _CACHE:
        _CACHE["nc"] = _build_nc()
    return _CACHE["nc"]


def _prep_weights(A, B):
    A = np.asarray(A, dtype=np.float32)
    B = np.asarray(B, dtype=np.float32)
    at = np.empty((2, NCD, 128, 128), dtype=np.float32)
    bt = np.empty((2, 128, O), dtype=np.float32)
    for p in range(2):
        # stationary for GEMM1: [D, 128] with expert 2p in cols 0-63, 2p+1 in 64-127
        atp = np.concatenate([A[2 * p].T, A[2 * p + 1].T], axis=1)  # [4096, 128]
        at[p] = atp.reshape(NCD, 128, 128)
        # moving for GEMM2: [128, O] with expert 2p rows 0-63, 2p+1 rows 64-127
        bt[p] = np.concatenate([B[2 * p].T, B[2 * p + 1].T], axis=0)
    return at, bt


def kernel(x, A, B, _trace=False):
    x = np.asarray(x, dtype=np.float32)
    xt_full = np.ascontiguousarray(x.reshape(T, D).T)  # [D, T]
    at, bt = _prep_weights(A, B)

    nc = _get_nc()
    in_maps = [
        {
            "xT": np.ascontiguousarray(xt_full[:, k * TL : (k + 1) * TL]),
            "AT": at,
            "BT": bt,
        }
        for k in range(NCORES)
    ]
    res = run_bass_kernel_spmd(nc, in_maps, list(range(NCORES)), trace=_trace)
    if _trace:
        _CACHE["last_result"] = res

    y = np.empty((E, T, O), dtype=np.float32)
    for k in range(NCORES):
        y[:, k * TL : (k + 1) * TL, :] = res.results[k]["y"]
    return y.reshape(E, B_DIM, S, O)
